# revision 48
# baseline (speedup 1.0000x reference)
"""
Trainium2 Bass kernel for nn_Attention_6150393168649.

Transformer-XL-style relative-position attention, b=16 t=512 d=256 h=4 hd=64,
MAX_REL=30.  Data-parallel over batch across 8 NeuronCores (2 batches/core);
weights replicated.

Key algorithmic points (per core):
  - LayerNorm stats in token-major layout; gamma/beta application folded into
    the PSUM->SBUF epilogue of the x transposes.
  - All linears consume xT (features on partitions); weights arrive
    host-pre-transposed and pre-tiled (layout-only marshalling).
  - rel-pos: the [t,t,d] tensor has only 61 distinct rows ->
    posT = (enc @ Wpos.T).T projected on device (61 x 256).
    pos scores factor through exp:  attn = exp(qk/8) * m, with
    m[t,s] = exp((pos[t,clip(s-t)+30] - pos[t,0])/8)  (the per-row constant
    pos[t,0] is dropped -- softmax invariant).  m == 1 left of the band;
    m == w[t] right of the band; only a 61-wide band is nontrivial.
  - The diagonal band "skew" runs through a DRAM scratch: each row stores
    [left-pad=1.0 | exp-multiplier band x61 | right-pad=w[t]] at stride 318;
    a stride-317 strided read yields m[t, s] for a 188-wide strip per
    128-token chunk.  Clip values come free from the pads.
  - softmax denominators come free from a ones-column appended to v
    (contexts computed unnormalized, divided in ctxT space via a rank-1
    broadcast matmul of the reciprocal).
  - attn is transposed for the context matmul with PE transposes (bf16).
  - matmuls run in float32r (TensorE full rate at N>=256); the attention
    probability path (attn/m tables) is bf16.
"""

import math
import sys

import numpy as np

sys.path.insert(0, "/opt/trn_rl_repo")

import concourse.bass as bass  # noqa: E402
import concourse.mybir as mybir  # noqa: E402
import concourse.tile as tile  # noqa: E402
from concourse import bacc as _bacc  # noqa: E402
from concourse.bass_utils import run_bass_kernel_spmd  # noqa: E402

# Problem constants (hardcoded per instructions)
B = 16
T = 512
D = 256
H = 4
HD = 64
MAX_REL = 30
NR = 2 * MAX_REL + 1  # 61
NCORES = 8
BPC = B // NCORES  # batches per core
N = BPC * T  # local tokens per core (1024)
P = 128

# skew buffer geometry: row = [left-pad(128) | band(61) | right-pad(128) | 1]
PAD = 128
ROWW = PAD + NR + PAD + 1  # 318
NSKEW = 4  # rotating skew buffers

FP = mybir.dt.float32
FR = mybir.dt.float32r
BF = mybir.dt.bfloat16

# consts block column offsets (in fp32 words per partition)
W_OFF = {"wqT": 0, "wkT": 512, "wvT": 1024, "woT": 1536, "wposT": 2048}
ENC_OFF = 2560  # [2, 64] -> 128 (61 used, zero-padded for even-N f32r)
ID_OFF = 2688  # [128]
VEC_OFF = 2816  # [2, 8] -> 16
ONES_OFF = 2832  # [128] row of ones (row 0)
CONSTW = 2960

_CACHE = {}


def _enc_table():
    """61 x 256 sinusoidal table over clipped relative distances (pure
    function of (t, d); mirrors reference._rel_pos_encodings rows)."""
    n = NR
    positions = np.arange(n, dtype=np.float32)[:, None]
    div_term = np.exp(
        np.arange(0, D, 2, dtype=np.float32) * (-math.log(10000.0) / D)
    )
    ang = positions * div_term  # [n, d/2]
    enc = np.stack([np.sin(ang), np.cos(ang)], axis=-1).reshape(n, D)
    return enc.astype(np.float32)  # [61, 256]


def _build_nc():
    # Bacc (not raw Bass): its compile() legalizes multi-wait instructions
    # into standalone event-semaphores (the raw ISA has one wait slot).
    nc = _bacc.Bacc(
        "TRN2", target_bir_lowering=False, debug=False, num_devices=NCORES
    )

    x_ext = nc.declare_dram_parameter("x", [N, D], FP, isOutput=False)
    c_ext = nc.declare_dram_parameter("consts", [P, CONSTW], FR, isOutput=False)
    r_ext = nc.declare_dram_parameter("rows", [2, D], FR, isOutput=False)
    out_ext = nc.declare_dram_parameter("out", [N, D], FP, isOutput=True)

    skew = nc.dram_tensor("skewbuf", [NSKEW, T, ROWW], BF)

    EXP = mybir.ActivationFunctionType.Exp
    SQRT = mybir.ActivationFunctionType.Sqrt
    MUL = mybir.AluOpType.mult
    ADD = mybir.AluOpType.add
    SUB = mybir.AluOpType.subtract
    AXX = mybir.AxisListType.X

    with nc.allow_low_precision(
        reason="float32r matmuls (32-bit) + bf16 attention probabilities"
    ), tile.TileContext(nc) as tc:
        with (
            tc.tile_pool(name="persist", bufs=1) as pers,
            tc.tile_pool(name="work", bufs=3) as work,
            tc.tile_pool(name="xcp", bufs=1) as xcp,
            tc.tile_pool(name="attnp", bufs=8) as attnp,
            tc.tile_pool(name="attntp", bufs=8) as attntp,
            tc.tile_pool(name="mtabp", bufs=4) as mtabp,
            tc.tile_pool(name="small", bufs=6) as small,
            tc.tile_pool(name="psA", bufs=2, space="PSUM") as psA,
            tc.tile_pool(name="psB", bufs=2, space="PSUM") as psB,
            tc.tile_pool(name="psP", bufs=2, space="PSUM") as psP,
            tc.tile_pool(name="psD", bufs=1, space="PSUM") as psD,
            tc.tile_pool(name="psC", bufs=1, space="PSUM") as psC,
        ):
            # ------------- constants (one DMA per weight block) -------------
            def wload(name):
                t_ = pers.tile([P, 2, D], FR, tag=f"w_{name}")
                nc.sync.dma_start(
                    out=t_,
                    in_=c_ext[:, W_OFF[name] : W_OFF[name] + 512].rearrange(
                        "p (c o) -> p c o", c=2
                    ),
                )
                return t_

            wq_sb = wload("wqT")
            wk_sb = wload("wkT")
            wv_sb = wload("wvT")
            wo_sb = wload("woT")
            wp_sb = wload("wposT")
            tail = pers.tile([P, CONSTW - ENC_OFF], FR, tag="ctail")
            nc.sync.dma_start(out=tail, in_=c_ext[:, ENC_OFF:])
            enc_sb = tail[:, 0:128].rearrange("p (c r) -> p c r", c=2)
            ident_sb = tail[:, ID_OFF - ENC_OFF : ID_OFF - ENC_OFF + 128]
            vecs_sb = tail[
                :, VEC_OFF - ENC_OFF : VEC_OFF - ENC_OFF + 16
            ].rearrange("p (c k) -> p c k", c=2)
            # vec k: 0 gamma, 1 beta, 2 bq, 3 bk, 4 bpos, 5 ub, 6 vb, 7 spare
            # (copy to a plain-fp32 tile: tensor_scalar wants fp32 scalars)
            vecs_fp = pers.tile([P, 2, 8], FP, tag="vecs_fp")
            nc.vector.tensor_copy(out=vecs_fp, in_=vecs_sb)
            gamma_col = vecs_fp[:, :, 0]
            beta_col = vecs_fp[:, :, 1]
            bk_col = vecs_fp[:, :, 3]
            bp_col = vecs_fp[:, :, 4]

            rows_sb = pers.tile([1, 2, D], FR, tag="rows")
            nc.sync.dma_start(
                out=rows_sb, in_=r_ext[:].rearrange("(o r) d -> o r d", o=1)
            )
            bv_row = rows_sb[:, 0, :]
            bo_row = rows_sb[:, 1, :]

            ident_bf = pers.tile([P, P], BF, tag="ident_bf")
            nc.vector.tensor_copy(out=ident_bf, in_=ident_sb)
            ones_row = tail[0:1, ONES_OFF - ENC_OFF : ONES_OFF - ENC_OFF + P]
            ones_pad = pers.tile([P, PAD], BF, tag="ones_pad")
            nc.vector.memset(ones_pad, 1.0)
            eps_t = pers.tile([P, 1], FP, tag="eps")
            nc.vector.memset(eps_t, 1e-5)

            bqu_col = pers.tile([P, 2], FP, tag="bqu")
            nc.vector.tensor_tensor(
                bqu_col, vecs_fp[:, :, 2], vecs_fp[:, :, 5], ADD
            )
            bqv_col = pers.tile([P, 2], FP, tag="bqv")
            nc.vector.tensor_tensor(
                bqv_col, vecs_fp[:, :, 2], vecs_fp[:, :, 6], ADD
            )

            # prefill left pads of the skew buffers with 1.0 (once each)
            for bi in range(NSKEW):
                dst = bass.AP(
                    tensor=skew[bi].tensor,
                    offset=skew[bi].offset,
                    ap=[[ROWW, P], [ROWW * P, 4], [1, PAD]],
                )
                src = bass.AP(
                    tensor=ones_pad.tensor,
                    offset=ones_pad[:].offset,
                    ap=[list(ones_pad[:].ap[0]), [0, 4], [1, PAD]],
                )
                nc.sync.dma_start(out=dst, in_=src)

            # ---------------- load x (one DMA) + LayerNorm ----------------
            x_all = pers.tile([P, N // P, D], FP, tag="x_all")
            nc.sync.dma_start(
                out=x_all, in_=x_ext[:].rearrange("(j p) d -> p j d", p=P)
            )
            nj = N // P
            s1 = small.tile([P, nj], FP, tag="s1")
            nc.vector.reduce_sum(out=s1, in_=x_all, axis=AXX)
            mu = small.tile([P, nj], FP, tag="mu")
            nc.vector.tensor_scalar_mul(mu, s1, 1.0 / D)
            xhat = xcp.tile([P, nj, D], FR, tag="xc")
            nc.vector.tensor_tensor(
                xhat, x_all, mu[:, :, None].to_broadcast(x_all.shape), SUB
            )
            sq = xcp.tile([P, nj, D], FP, tag="sq")
            var = small.tile([P, nj], FP, tag="var")
            for j in range(nj):
                nc.scalar.activation(
                    out=sq[:, j, :],
                    in_=xhat[:, j, :],
                    func=mybir.ActivationFunctionType.Square,
                    accum_out=var[:, j : j + 1],
                )
            std = small.tile([P, nj], FP, tag="std")
            nc.scalar.activation(
                out=std, in_=var, func=SQRT, bias=eps_t[:, 0:1],
                scale=1.0 / D,
            )
            rs = small.tile([P, nj], FP, tag="rs")
            nc.vector.reciprocal(out=rs, in_=std)
            nc.vector.tensor_tensor(
                xhat, xhat, rs[:, :, None].to_broadcast(xhat.shape), MUL
            )
            xhat_tiles = [xhat[:, j, :] for j in range(nj)]

            # -------- transpose x -> xT (gamma/beta in the epilogue) --------
            xT = pers.tile([P, 2, N], FR, tag="xT")
            for c in range(2):
                for g in range(2):
                    ps = psA.tile([P, 512], FR, tag="psA")
                    for jj in range(4):
                        j = 4 * g + jj
                        nc.tensor.transpose(
                            ps[:, P * jj : P * (jj + 1)],
                            xhat_tiles[j][:, P * c : P * (c + 1)],
                            ident_sb,
                        )
                    nc.vector.tensor_scalar(
                        out=xT[:, c, 512 * g : 512 * (g + 1)],
                        in0=ps,
                        scalar1=gamma_col[:, c : c + 1],
                        scalar2=beta_col[:, c : c + 1],
                        op0=MUL,
                        op1=ADD,
                    )

            # ---------------- projections ----------------
            qTu = pers.tile([P, 2, N], FR, tag="qTu")
            qTv = pers.tile([P, 2, N], FR, tag="qTv")
            kT = pers.tile([P, 2, N], FR, tag="kT")
            for oc in range(2):
                for g in range(2):
                    psq = psA.tile([P, 512], FP, tag="psA")
                    psk = psB.tile([P, 512], FP, tag="psB")
                    for kc in range(2):
                        nc.tensor.matmul(
                            psq,
                            lhsT=wq_sb[:, kc, P * oc : P * (oc + 1)],
                            rhs=xT[:, kc, 512 * g : 512 * (g + 1)],
                            start=(kc == 0),
                            stop=(kc == 1),
                        )
                        nc.tensor.matmul(
                            psk,
                            lhsT=wk_sb[:, kc, P * oc : P * (oc + 1)],
                            rhs=xT[:, kc, 512 * g : 512 * (g + 1)],
                            start=(kc == 0),
                            stop=(kc == 1),
                        )
                    sl = (slice(None), oc, slice(512 * g, 512 * (g + 1)))
                    nc.vector.tensor_scalar_add(
                        qTu[sl], psq, bqu_col[:, oc : oc + 1]
                    )
                    nc.scalar.activation(
                        out=qTv[sl],
                        in_=psq,
                        func=mybir.ActivationFunctionType.Identity,
                        bias=bqv_col[:, oc : oc + 1],
                    )
                    nc.vector.tensor_scalar_add(
                        kT[sl], psk, bk_col[:, oc : oc + 1]
                    )

            # v in token-major layout with a ones column per head (bf16)
            v_sb = pers.tile([P, N // P, H, HD + 1], BF, tag="v_sb")
            nc.vector.memset(v_sb, 1.0)  # ones cols (rest overwritten)
            for j in range(N // P):
                psv = psB.tile([P, 512], FP, tag="psB")
                for kc in range(2):
                    nc.tensor.matmul(
                        psv[:, 0:D],
                        lhsT=xT[:, kc, P * j : P * (j + 1)],
                        rhs=wv_sb[:, kc, :],
                        start=(kc == 0),
                        stop=False,
                    )
                nc.tensor.matmul(
                    psv[:, 0:D],
                    lhsT=ones_row,
                    rhs=bv_row,
                    start=False,
                    stop=True,
                )
                nc.vector.tensor_copy(
                    out=v_sb[:, j, :, 0:HD],
                    in_=psv[:, 0:D].rearrange("p (h d) -> p h d", h=H),
                )

            # ---------------- posT = (enc @ Wpos.T).T + bpos ----------------
            posT = pers.tile([P, 2, 64], FR, tag="posT")
            for mc in range(2):
                psp = psP.tile([P, 4, 64], FP, tag="psP")
                for kc in range(2):
                    nc.tensor.matmul(
                        psp[:, 0, :],
                        lhsT=wp_sb[:, kc, P * mc : P * (mc + 1)],
                        rhs=enc_sb[:, kc, :],
                        start=(kc == 0),
                        stop=(kc == 1),
                    )
                nc.vector.tensor_scalar_add(
                    posT[:, mc, :], psp[:, 0, :], bp_col[:, mc : mc + 1]
                )
            # delta table: pos_r - pos_0 -> m = exp(qv . dpos / 8), no bias
            posTd = pers.tile([P, 2, 64], FR, tag="posTd")
            nc.vector.tensor_tensor(
                posTd,
                posT,
                posT[:, :, 0:1].to_broadcast(posT.shape),
                SUB,
            )

            # ---------------- attention per (batch, head-pair) ----------------
            # Heads 2*hp and 2*hp+1 live on partition halves [0:64) / [64:128)
            # of chunk hp; interleaving their K=64 matmuls back-to-back lets
            # the PE run them concurrently in separate row-groups.
            ctxT = pers.tile([P, 2, N], FR, tag="ctxT")
            for b in range(BPC):
                for hp in range(2):
                    oc = hp
                    tb = T * b

                    # --- p-matmuls + m-tables + skew, per head (interleaved MMs) ---
                    psps, mtabs, wvalss, strips = [], [], [], []
                    for hh in range(2):
                        psp_h = psP.tile([P, 4, 64], FP, tag="psP", name=f"psp_{hh}")
                        psps.append(psp_h)
                    for t4 in range(4):
                        for hh in range(2):
                            po = HD * hh
                            nc.tensor.matmul(
                                psps[hh][:, t4, :],
                                lhsT=qTv[
                                    po : po + HD,
                                    oc,
                                    tb + P * t4 : tb + P * (t4 + 1),
                                ],
                                rhs=posTd[po : po + HD, oc, :],
                                start=True,
                                stop=True,
                            )
                    for hh in range(2):
                        bh = b * H + 2 * hp + hh
                        bi = bh % NSKEW
                        mtab = mtabp.tile([P, 4, 64], BF, tag="mtab")
                        wrep = mtabp.tile([P, 4, PAD], BF, tag="wrep")
                        wvals = small.tile([P, 4], FP, tag="wvals")
                        nc.scalar.activation(
                            out=mtab[:], in_=psps[hh][:], func=EXP, scale=0.125
                        )
                        nc.gpsimd.tensor_copy(
                            out=wvals, in_=mtab[:, :, NR - 1]
                        )
                        for t4 in range(4):
                            nc.gpsimd.tensor_scalar_mul(
                                wrep[:, t4, :], ones_pad, wvals[:, t4 : t4 + 1]
                            )
                        nc.sync.dma_start(
                            out=bass.AP(
                                tensor=skew[bi].tensor,
                                offset=skew[bi].offset + PAD,
                                ap=[[ROWW, P], [ROWW * P, 4], [1, NR]],
                            ),
                            in_=mtab[:, :, 0:NR],
                        )
                        nc.sync.dma_start(
                            out=bass.AP(
                                tensor=skew[bi].tensor,
                                offset=skew[bi].offset + PAD + NR,
                                ap=[[ROWW, P], [ROWW * P, 4], [1, PAD]],
                            ),
                            in_=wrep[:],
                        )
                        strip = work.tile([P, 4, 188], BF, tag="strip")
                        nc.sync.dma_start(
                            out=strip,
                            in_=bass.AP(
                                tensor=skew[bi].tensor,
                                offset=skew[bi].offset + PAD,
                                ap=[[ROWW - 1, P], [ROWW * P, 4], [1, 188]],
                            ),
                        )
                        mtabs.append(mtab)
                        wvalss.append(wvals)
                        strips.append(strip)

                    # --- scores -> exp -> multiplier (pair-interleaved) ---
                    attns = [[], []]
                    for t4 in range(4):
                        T0 = P * t4
                        s_lo = max(0, T0 - 30)
                        s_hi = min(T, T0 + 30 + P)
                        c_lo = s_lo - (T0 - 30)
                        psss = []
                        for hh in range(2):
                            po = HD * hh
                            pss = psA.tile([P, 512], FP, tag="psA")
                            nc.tensor.matmul(
                                pss,
                                lhsT=qTu[
                                    po : po + HD,
                                    oc,
                                    tb + P * t4 : tb + P * (t4 + 1),
                                ],
                                rhs=kT[po : po + HD, oc, tb : tb + T],
                                start=True,
                                stop=True,
                            )
                            psss.append(pss)
                        for hh in range(2):
                            at = attnp.tile([P, T], BF, tag="attn")
                            nc.scalar.activation(
                                out=at, in_=psss[hh], func=EXP, scale=0.125
                            )
                            nc.vector.tensor_tensor(
                                at[:, s_lo:s_hi],
                                at[:, s_lo:s_hi],
                                strips[hh][:, t4, c_lo : c_lo + (s_hi - s_lo)],
                                MUL,
                            )
                            if s_hi < T:
                                nc.gpsimd.tensor_scalar_mul(
                                    at[:, s_hi:T],
                                    at[:, s_hi:T],
                                    wvalss[hh][:, t4 : t4 + 1],
                                )
                            attns[hh].append(at)

                    # --- transpose + context + normalize, per head ---
                    for hh in range(2):
                        h = 2 * hp + hh
                        po = HD * hh
                        attnTs = []
                        for s4 in range(4):
                            psat = psB.tile([P, 512], BF, tag="psB")
                            for t4 in range(4):
                                nc.tensor.transpose(
                                    psat[:, P * t4 : P * (t4 + 1)],
                                    attns[hh][t4][:, P * s4 : P * (s4 + 1)],
                                    ident_bf,
                                )
                            atT = attntp.tile([P, T], BF, tag="attnT")
                            nc.vector.tensor_copy(out=atT, in_=psat)
                            attnTs.append(atT)

                        psc = psC.tile([P, 512], FP, tag="psC")
                        for s4 in range(4):
                            j = 4 * b + s4
                            nc.tensor.matmul(
                                psc[0 : HD + 1, :],
                                lhsT=v_sb[:, j, h, :],
                                rhs=attnTs[s4],
                                start=(s4 == 0),
                                stop=(s4 == 3),
                            )
                        rden = small.tile([1, T], FR, tag="rden")
                        nc.vector.reciprocal(
                            out=rden, in_=psc[HD : HD + 1, :]
                        )
                        psd = psD.tile([P, 512], FP, tag="psD")
                        nc.tensor.matmul(
                            psd[0:HD, :],
                            lhsT=ones_row[0:1, 0:HD],
                            rhs=rden,
                            start=True,
                            stop=True,
                        )
                        denb = attntp.tile([HD, T], FP, tag="denb")
                        nc.vector.tensor_copy(out=denb, in_=psd[0:HD, :])
                        nc.vector.tensor_tensor(
                            ctxT[po : po + HD, oc, tb : tb + T],
                            psc[0:HD, :],
                            denb,
                            MUL,
                        )

            # ---------------- output projection ----------------
            out_all = pers.tile([P, N // P, D], FP, tag="out_all")
            for j in range(N // P):
                pso = psB.tile([P, 512], FP, tag="psB")
                for kc in range(2):
                    nc.tensor.matmul(
                        pso[:, 0:D],
                        lhsT=ctxT[:, kc, P * j : P * (j + 1)],
                        rhs=wo_sb[:, kc, :],
                        start=(kc == 0),
                        stop=False,
                    )
                nc.tensor.matmul(
                    pso[:, 0:D],
                    lhsT=ones_row,
                    rhs=bo_row,
                    start=False,
                    stop=True,
                )
                nc.vector.tensor_copy(out=out_all[:, j, :], in_=pso[:, 0:D])
            nc.sync.dma_start(
                out=out_ext[:].rearrange("(j p) d -> p j d", p=P), in_=out_all
            )

    nc.finalize()
    return nc


def _get_nc():
    if "nc" not in _CACHE:
        _CACHE["nc"] = _build_nc()
    return _CACHE["nc"]


def _make_in_maps(inputs):
    x = np.asarray(inputs["inputs"], dtype=np.float32)  # [16, 512, 256]
    enc = _enc_table()

    def wtile(w):
        # W [o, i] -> W.T [i, o] -> [p, (c o)] with i = c*128 + p
        return (
            np.asarray(w, np.float32)
            .T.reshape(2, P, D)
            .transpose(1, 0, 2)
            .reshape(P, 512)
        )

    def coltile(v):
        return np.asarray(v, np.float32).reshape(2, P).T  # [p, c]

    consts = np.zeros((P, CONSTW), np.float32)
    for name, w in [
        ("wqT", inputs["Wq"]),
        ("wkT", inputs["Wk"]),
        ("wvT", inputs["Wv"]),
        ("woT", inputs["Wo"]),
        ("wposT", inputs["Wpos"]),
    ]:
        consts[:, W_OFF[name] : W_OFF[name] + 512] = wtile(w)
    encp = np.zeros((2, P, 64), np.float32)
    encp[:, :, 0:NR] = enc.T.reshape(2, P, NR)
    consts[:, ENC_OFF : ENC_OFF + 128] = encp.transpose(1, 0, 2).reshape(P, 128)
    consts[:, ID_OFF : ID_OFF + 128] = np.eye(P, dtype=np.float32)
    consts[0, ONES_OFF : ONES_OFF + P] = 1.0
    vecs = np.zeros((P, 2, 8), np.float32)
    vecs[:, :, 0] = coltile(inputs["ln_gamma"])
    vecs[:, :, 1] = coltile(inputs["ln_beta"])
    vecs[:, :, 2] = coltile(inputs["bq"])
    vecs[:, :, 3] = coltile(inputs["bk"])
    vecs[:, :, 4] = coltile(inputs["bpos"])
    vecs[:, :, 5] = coltile(np.asarray(inputs["u_bias"], np.float32).reshape(D))
    vecs[:, :, 6] = coltile(np.asarray(inputs["v_bias"], np.float32).reshape(D))
    consts[:, VEC_OFF : VEC_OFF + 16] = vecs.reshape(P, 16)

    rows = np.stack(
        [
            np.asarray(inputs["bv"], np.float32),
            np.asarray(inputs["bo"], np.float32),
        ]
    )
    common = {
        "consts": np.ascontiguousarray(consts),
        "rows": np.ascontiguousarray(rows),
    }
    in_maps = []
    for core in range(NCORES):
        m = dict(common)
        m["x"] = np.ascontiguousarray(
            x[BPC * core : BPC * (core + 1)].reshape(N, D)
        )
        in_maps.append(m)
    return in_maps


def run(inputs, trace=False):
    nc = _get_nc()
    in_maps = _make_in_maps(inputs)
    res = run_bass_kernel_spmd(
        nc, in_maps, core_ids=list(range(NCORES)), trace=trace
    )
    outs = [np.asarray(r["out"]) for r in res.results]
    full = np.concatenate(outs, axis=0).reshape(B, T, D).astype(np.float32)
    return full, res


def kernel(**inputs) -> np.ndarray:
    full, _ = run(inputs, trace=False)
    return full


# revision 56
# speedup vs baseline: 1.1842x; 1.1842x over previous
"""
Trainium2 Bass kernel for nn_Attention_6150393168649.

Transformer-XL-style relative-position attention, b=16 t=512 d=256 h=4 hd=64,
MAX_REL=30.  Data-parallel over batch across 8 NeuronCores (2 batches/core);
weights replicated.

Key algorithmic points (per core):
  - LayerNorm stats in token-major layout; gamma/beta application folded into
    the PSUM->SBUF epilogue of the x transposes.
  - All linears consume xT (features on partitions); weights arrive
    host-pre-transposed and pre-tiled (layout-only marshalling).
  - rel-pos: the [t,t,d] tensor has only 61 distinct rows ->
    posT = (enc @ Wpos.T).T projected on device (61 x 256).
    pos scores factor through exp:  attn = exp(qk/8) * m, with
    m[t,s] = exp((pos[t,clip(s-t)+30] - pos[t,0])/8)  (the per-row constant
    pos[t,0] is dropped -- softmax invariant).  m == 1 left of the band;
    m == w[t] right of the band; only a 61-wide band is nontrivial.
  - The diagonal band "skew" runs through a DRAM scratch: each row stores
    [left-pad=1.0 | exp-multiplier band x61 | right-pad=w[t]] at stride 318;
    a stride-317 strided read yields m[t, s] for a 188-wide strip per
    128-token chunk.  Clip values come free from the pads.
  - softmax denominators come free from a ones-column appended to v
    (contexts computed unnormalized, divided in ctxT space via a rank-1
    broadcast matmul of the reciprocal).
  - attn is transposed for the context matmul with PE transposes (bf16).
  - matmuls run in float32r (TensorE full rate at N>=256); the attention
    probability path (attn/m tables) is bf16.
"""

import math
import sys

import numpy as np

sys.path.insert(0, "/opt/trn_rl_repo")

import concourse.bass as bass  # noqa: E402
import concourse.mybir as mybir  # noqa: E402
import concourse.tile as tile  # noqa: E402
from concourse import bacc as _bacc  # noqa: E402
from concourse.bass_utils import run_bass_kernel_spmd  # noqa: E402

# Problem constants (hardcoded per instructions)
B = 16
T = 512
D = 256
H = 4
HD = 64
MAX_REL = 30
NR = 2 * MAX_REL + 1  # 61
NCORES = 8
BPC = B // NCORES  # batches per core
N = BPC * T  # local tokens per core (1024)
P = 128

# skew buffer geometry: row = [left-pad(128) | band(61) | right-pad(128) | 1]
PAD = 128
ROWW = PAD + NR + PAD + 1  # 318
NSKEW = 8  # rotating skew buffers (one per (batch, head): no WAR reuse)

FP = mybir.dt.float32
FR = mybir.dt.float32r
BF = mybir.dt.bfloat16

# consts block column offsets (in fp32 words per partition)
W_OFF = {"wqT": 0, "wkT": 512, "wvT": 1024, "woT": 1536, "wposT": 2048}
ENC_OFF = 2560  # [2, 64] -> 128 (61 used, zero-padded for even-N f32r)
ID_OFF = 2688  # [128]
VEC_OFF = 2816  # [2, 8] -> 16
ONES_OFF = 2832  # [128] row of ones (row 0)
CONSTW = 2960

_CACHE = {}


def _enc_table():
    """61 x 256 sinusoidal table over clipped relative distances (pure
    function of (t, d); mirrors reference._rel_pos_encodings rows)."""
    n = NR
    positions = np.arange(n, dtype=np.float32)[:, None]
    div_term = np.exp(
        np.arange(0, D, 2, dtype=np.float32) * (-math.log(10000.0) / D)
    )
    ang = positions * div_term  # [n, d/2]
    enc = np.stack([np.sin(ang), np.cos(ang)], axis=-1).reshape(n, D)
    return enc.astype(np.float32)  # [61, 256]


def _build_nc():
    # Bacc (not raw Bass): its compile() legalizes multi-wait instructions
    # into standalone event-semaphores (the raw ISA has one wait slot).
    nc = _bacc.Bacc(
        "TRN2", target_bir_lowering=False, debug=False, num_devices=NCORES
    )

    x_ext = nc.declare_dram_parameter("x", [N, D], FP, isOutput=False)
    c_ext = nc.declare_dram_parameter("consts", [P, CONSTW], FR, isOutput=False)
    r_ext = nc.declare_dram_parameter("rows", [2, D], FR, isOutput=False)
    out_ext = nc.declare_dram_parameter("out", [N, D], FP, isOutput=True)

    skew = nc.dram_tensor("skewbuf", [NSKEW, T, ROWW], BF)

    EXP = mybir.ActivationFunctionType.Exp
    SQRT = mybir.ActivationFunctionType.Sqrt
    MUL = mybir.AluOpType.mult
    ADD = mybir.AluOpType.add
    SUB = mybir.AluOpType.subtract
    AXX = mybir.AxisListType.X

    with nc.allow_low_precision(
        reason="float32r matmuls (32-bit) + bf16 attention probabilities"
    ), tile.TileContext(nc) as tc:
        with (
            tc.tile_pool(name="persist", bufs=1) as pers,
            tc.tile_pool(name="work", bufs=4) as work,
            tc.tile_pool(name="xcp", bufs=1) as xcp,
            tc.tile_pool(name="attnp", bufs=8) as attnp,
            tc.tile_pool(name="attntp", bufs=10) as attntp,
            tc.tile_pool(name="mtabp", bufs=4) as mtabp,
            tc.tile_pool(name="small", bufs=8) as small,
            tc.tile_pool(name="psA", bufs=2, space="PSUM") as psA,
            tc.tile_pool(name="psB", bufs=3, space="PSUM") as psB,
            tc.tile_pool(name="psP", bufs=1, space="PSUM") as psP,
            tc.tile_pool(name="psD", bufs=1, space="PSUM") as psD,
            tc.tile_pool(name="psC", bufs=1, space="PSUM") as psC,
        ):
            # ------------- constants (one DMA per weight block) -------------
            def wload(name):
                t_ = pers.tile([P, 2, D], FR, tag=f"w_{name}")
                nc.sync.dma_start(
                    out=t_,
                    in_=c_ext[:, W_OFF[name] : W_OFF[name] + 512].rearrange(
                        "p (c o) -> p c o", c=2
                    ),
                )
                return t_

            tail = pers.tile([P, CONSTW - ENC_OFF], FR, tag="ctail")
            nc.sync.dma_start(out=tail, in_=c_ext[:, ENC_OFF:])
            enc_sb = tail[:, 0:128].rearrange("p (c r) -> p c r", c=2)
            ident_sb = tail[:, ID_OFF - ENC_OFF : ID_OFF - ENC_OFF + 128]
            vecs_sb = tail[
                :, VEC_OFF - ENC_OFF : VEC_OFF - ENC_OFF + 16
            ].rearrange("p (c k) -> p c k", c=2)
            # vec k: 0 gamma, 1 beta, 2 bq, 3 bk, 4 bpos, 5 ub, 6 vb, 7 spare
            # (copy to a plain-fp32 tile: tensor_scalar wants fp32 scalars)
            vecs_fp = pers.tile([P, 2, 8], FP, tag="vecs_fp")
            nc.vector.tensor_copy(out=vecs_fp, in_=vecs_sb)
            gamma_col = vecs_fp[:, :, 0]
            beta_col = vecs_fp[:, :, 1]
            bk_col = vecs_fp[:, :, 3]
            bp_col = vecs_fp[:, :, 4]

            rows_sb = pers.tile([1, 2, D], FR, tag="rows")
            nc.sync.dma_start(
                out=rows_sb, in_=r_ext[:].rearrange("(o r) d -> o r d", o=1)
            )
            bv_row = rows_sb[:, 0, :]
            bo_row = rows_sb[:, 1, :]

            ident_bf = pers.tile([P, P], BF, tag="ident_bf")
            nc.vector.tensor_copy(out=ident_bf, in_=ident_sb)
            ones_row = tail[0:1, ONES_OFF - ENC_OFF : ONES_OFF - ENC_OFF + P]
            ones_pad = pers.tile([P, PAD], BF, tag="ones_pad")
            nc.vector.memset(ones_pad, 1.0)
            eps_t = pers.tile([P, 1], FP, tag="eps")
            nc.vector.memset(eps_t, 1e-5)

            bqu_col = pers.tile([P, 2], FP, tag="bqu")
            nc.vector.tensor_tensor(
                bqu_col, vecs_fp[:, :, 2], vecs_fp[:, :, 5], ADD
            )
            bqv_col = pers.tile([P, 2], FP, tag="bqv")
            nc.vector.tensor_tensor(
                bqv_col, vecs_fp[:, :, 2], vecs_fp[:, :, 6], ADD
            )

            # ------------- load x (two DMAs) + LayerNorm (two halves) -------
            nj = N // P
            hj = nj // 4
            x_all = pers.tile([P, nj, D], FP, tag="x_all")
            xhat = xcp.tile([P, nj, D], FR, tag="xc")
            sq = xcp.tile([P, nj, D], FP, tag="sq")
            xv = x_ext[:].rearrange("(j p) d -> p j d", p=P)
            for g in range(4):
                gs = slice(hj * g, hj * (g + 1))
                nc.sync.dma_start(out=x_all[:, gs, :], in_=xv[:, gs, :])
                s1 = small.tile([P, hj], FP, tag="s1")
                nc.vector.reduce_sum(out=s1, in_=x_all[:, gs, :], axis=AXX)
                mu = small.tile([P, hj], FP, tag="mu")
                nc.vector.tensor_scalar_mul(mu, s1, 1.0 / D)
                nc.vector.tensor_tensor(
                    xhat[:, gs, :],
                    x_all[:, gs, :],
                    mu[:, :, None].to_broadcast((P, hj, D)),
                    SUB,
                )
                var = small.tile([P, hj], FP, tag="var")
                for j in range(hj):
                    nc.scalar.activation(
                        out=sq[:, hj * g + j, :],
                        in_=xhat[:, hj * g + j, :],
                        func=mybir.ActivationFunctionType.Square,
                        accum_out=var[:, j : j + 1],
                    )
                std = small.tile([P, hj], FP, tag="std")
                nc.scalar.activation(
                    out=std, in_=var, func=SQRT, bias=eps_t[:, 0:1],
                    scale=1.0 / D,
                )
                rs = small.tile([P, hj], FP, tag="rs")
                nc.vector.reciprocal(out=rs, in_=std)
                nc.vector.tensor_tensor(
                    xhat[:, gs, :],
                    xhat[:, gs, :],
                    rs[:, :, None].to_broadcast((P, hj, D)),
                    MUL,
                )
            xhat_tiles = [xhat[:, j, :] for j in range(nj)]

            wq_sb = wload("wqT")
            wk_sb = wload("wkT")
            wv_sb = wload("wvT")
            wo_sb = wload("woT")
            wp_sb = wload("wposT")
            # prefill left pads of the skew buffers with 1.0 (once each)
            for bi in range(NSKEW):
                dst = bass.AP(
                    tensor=skew[bi].tensor,
                    offset=skew[bi].offset,
                    ap=[[ROWW, P], [ROWW * P, 4], [1, PAD]],
                )
                src = bass.AP(
                    tensor=ones_pad.tensor,
                    offset=ones_pad[:].offset,
                    ap=[list(ones_pad[:].ap[0]), [0, 4], [1, PAD]],
                )
                nc.sync.dma_start(out=dst, in_=src)

            # -------- transpose x -> xT (gamma/beta in the epilogue) --------
            xT = pers.tile([P, 2, N], FR, tag="xT")
            for c in range(2):
                for g in range(2):
                    ps = psA.tile([P, 512], FR, tag="psA")
                    for jj in range(4):
                        j = 4 * g + jj
                        nc.tensor.transpose(
                            ps[:, P * jj : P * (jj + 1)],
                            xhat_tiles[j][:, P * c : P * (c + 1)],
                            ident_sb,
                        )
                    nc.vector.tensor_scalar(
                        out=xT[:, c, 512 * g : 512 * (g + 1)],
                        in0=ps,
                        scalar1=gamma_col[:, c : c + 1],
                        scalar2=beta_col[:, c : c + 1],
                        op0=MUL,
                        op1=ADD,
                    )

            # ---------------- projections ----------------
            qTu = pers.tile([P, 2, N], FR, tag="qTu")
            qTv = pers.tile([P, 2, N], FR, tag="qTv")
            kT = pers.tile([P, 2, N], FR, tag="kT")
            for oc in range(2):
                for g in range(2):
                    psq = psA.tile([P, 512], FP, tag="psA")
                    psk = psB.tile([P, 512], FP, tag="psB")
                    for kc in range(2):
                        nc.tensor.matmul(
                            psq,
                            lhsT=wq_sb[:, kc, P * oc : P * (oc + 1)],
                            rhs=xT[:, kc, 512 * g : 512 * (g + 1)],
                            start=(kc == 0),
                            stop=(kc == 1),
                        )
                        nc.tensor.matmul(
                            psk,
                            lhsT=wk_sb[:, kc, P * oc : P * (oc + 1)],
                            rhs=xT[:, kc, 512 * g : 512 * (g + 1)],
                            start=(kc == 0),
                            stop=(kc == 1),
                        )
                    sl = (slice(None), oc, slice(512 * g, 512 * (g + 1)))
                    nc.vector.tensor_scalar_add(
                        qTu[sl], psq, bqu_col[:, oc : oc + 1]
                    )
                    nc.scalar.activation(
                        out=qTv[sl],
                        in_=psq,
                        func=mybir.ActivationFunctionType.Identity,
                        bias=bqv_col[:, oc : oc + 1],
                    )
                    nc.vector.tensor_scalar_add(
                        kT[sl], psk, bk_col[:, oc : oc + 1]
                    )

            # v in token-major layout with a ones column per head (bf16)
            v_sb = pers.tile([P, N // P, H, HD + 1], BF, tag="v_sb")
            nc.vector.memset(v_sb, 1.0)  # ones cols (rest overwritten)
            for j in range(N // P):
                psv = psB.tile([P, 512], FP, tag="psB")
                for kc in range(2):
                    nc.tensor.matmul(
                        psv[:, 0:D],
                        lhsT=xT[:, kc, P * j : P * (j + 1)],
                        rhs=wv_sb[:, kc, :],
                        start=(kc == 0),
                        stop=False,
                    )
                nc.tensor.matmul(
                    psv[:, 0:D],
                    lhsT=ones_row,
                    rhs=bv_row,
                    start=False,
                    stop=True,
                )
                nc.vector.tensor_copy(
                    out=v_sb[:, j, :, 0:HD],
                    in_=psv[:, 0:D].rearrange("p (h d) -> p h d", h=H),
                )

            # ---------------- posT = (enc @ Wpos.T).T + bpos ----------------
            posT = pers.tile([P, 2, 64], FR, tag="posT")
            for mc in range(2):
                psp = psP.tile([P, 4, 64], FP, tag="psP")
                for kc in range(2):
                    nc.tensor.matmul(
                        psp[:, 0, :],
                        lhsT=wp_sb[:, kc, P * mc : P * (mc + 1)],
                        rhs=enc_sb[:, kc, :],
                        start=(kc == 0),
                        stop=(kc == 1),
                    )
                nc.vector.tensor_scalar_add(
                    posT[:, mc, :], psp[:, 0, :], bp_col[:, mc : mc + 1]
                )
            # delta table: pos_r - pos_0 -> m = exp(qv . dpos / 8), no bias
            posTd = pers.tile([P, 2, 64], FR, tag="posTd")
            nc.vector.tensor_tensor(
                posTd,
                posT,
                posT[:, :, 0:1].to_broadcast(posT.shape),
                SUB,
            )

            # ------- m-tables + skew buffers for ALL heads (hoisted) -------
            # Only needs qTv/posTd; emitting all of it up front takes the
            # band-multiplier DMA round-trip off the attention critical path.
            all_wvals = {}
            all_strips = {}
            for b in range(BPC):
                for hp in range(2):
                    oc = hp
                    tb = T * b
                    psps = []
                    for hh in range(2):
                        psp_h = psP.tile([P, 4, 64], FP, tag="psP", name=f"psp_{b}_{hp}_{hh}")
                        psps.append(psp_h)
                    for t4 in range(4):
                        for hh in range(2):
                            po = HD * hh
                            nc.tensor.matmul(
                                psps[hh][:, t4, :],
                                lhsT=qTv[
                                    po : po + HD,
                                    oc,
                                    tb + P * t4 : tb + P * (t4 + 1),
                                ],
                                rhs=posTd[po : po + HD, oc, :],
                                start=True,
                                stop=True,
                            )
                    for hh in range(2):
                        bh = b * H + 2 * hp + hh
                        bi = bh % NSKEW
                        mtab = mtabp.tile([P, 4, 64], BF, tag="mtab")
                        wrep = mtabp.tile([P, 4, PAD], BF, tag="wrep")
                        wvals = small.tile([P, 4], FP, tag="wvals")
                        nc.scalar.activation(
                            out=mtab[:], in_=psps[hh][:], func=EXP, scale=0.125
                        )
                        nc.gpsimd.tensor_copy(
                            out=wvals, in_=mtab[:, :, NR - 1]
                        )
                        for t4 in range(4):
                            nc.gpsimd.tensor_scalar_mul(
                                wrep[:, t4, :], ones_pad, wvals[:, t4 : t4 + 1]
                            )
                        nc.sync.dma_start(
                            out=bass.AP(
                                tensor=skew[bi].tensor,
                                offset=skew[bi].offset + PAD,
                                ap=[[ROWW, P], [ROWW * P, 4], [1, NR]],
                            ),
                            in_=mtab[:, :, 0:NR],
                        )
                        nc.sync.dma_start(
                            out=bass.AP(
                                tensor=skew[bi].tensor,
                                offset=skew[bi].offset + PAD + NR,
                                ap=[[ROWW, P], [ROWW * P, 4], [1, PAD]],
                            ),
                            in_=wrep[:],
                        )
                        strip = work.tile(
                            [P, 4, 188], BF, tag=f"strip_{bh}", bufs=1
                        )
                        nc.sync.dma_start(
                            out=strip,
                            in_=bass.AP(
                                tensor=skew[bi].tensor,
                                offset=skew[bi].offset + PAD,
                                ap=[[ROWW - 1, P], [ROWW * P, 4], [1, 188]],
                            ),
                        )
                        all_wvals[bh] = wvals
                        all_strips[bh] = strip

            # ---------------- attention per (batch, head-pair) ----------------
            ctxT = pers.tile([P, 2, N], FR, tag="ctxT")
            for b in range(BPC):
                for hp in range(2):
                    oc = hp
                    tb = T * b
                    strips = [all_strips[b * H + 2 * hp + hh] for hh in range(2)]
                    wvalss = [all_wvals[b * H + 2 * hp + hh] for hh in range(2)]

                    # --- scores -> exp -> multiplier (pair-interleaved) ---
                    attns = [[], []]
                    for t4 in range(4):
                        T0 = P * t4
                        s_lo = max(0, T0 - 30)
                        s_hi = min(T, T0 + 30 + P)
                        c_lo = s_lo - (T0 - 30)
                        psss = []
                        for hh in range(2):
                            po = HD * hh
                            pss = psA.tile([P, 512], FP, tag="psA")
                            nc.tensor.matmul(
                                pss,
                                lhsT=qTu[
                                    po : po + HD,
                                    oc,
                                    tb + P * t4 : tb + P * (t4 + 1),
                                ],
                                rhs=kT[po : po + HD, oc, tb : tb + T],
                                start=True,
                                stop=True,
                            )
                            psss.append(pss)
                        for hh in range(2):
                            at = attnp.tile([P, T], BF, tag="attn")
                            nc.scalar.activation(
                                out=at, in_=psss[hh], func=EXP, scale=0.125
                            )
                            nc.vector.tensor_tensor(
                                at[:, s_lo:s_hi],
                                at[:, s_lo:s_hi],
                                strips[hh][:, t4, c_lo : c_lo + (s_hi - s_lo)],
                                MUL,
                            )
                            if s_hi < T:
                                nc.gpsimd.tensor_scalar_mul(
                                    at[:, s_hi:T],
                                    at[:, s_hi:T],
                                    wvalss[hh][:, t4 : t4 + 1],
                                )
                            attns[hh].append(at)

                    # --- transpose + context + normalize, per head ---
                    for hh in range(2):
                        h = 2 * hp + hh
                        po = HD * hh
                        attnTs = []
                        for s4 in range(4):
                            psat = psB.tile([P, 512], BF, tag="psB")
                            for t4 in range(4):
                                nc.tensor.transpose(
                                    psat[:, P * t4 : P * (t4 + 1)],
                                    attns[hh][t4][:, P * s4 : P * (s4 + 1)],
                                    ident_bf,
                                )
                            atT = attntp.tile([P, T], BF, tag="attnT")
                            nc.vector.tensor_copy(out=atT, in_=psat)
                            attnTs.append(atT)

                        psc = psC.tile([P, 512], FP, tag="psC")
                        for s4 in range(4):
                            j = 4 * b + s4
                            nc.tensor.matmul(
                                psc[0 : HD + 1, :],
                                lhsT=v_sb[:, j, h, :],
                                rhs=attnTs[s4],
                                start=(s4 == 0),
                                stop=(s4 == 3),
                            )
                        rden = small.tile([1, T], FR, tag="rden")
                        nc.vector.reciprocal(
                            out=rden, in_=psc[HD : HD + 1, :]
                        )
                        psd = psD.tile([P, 512], FP, tag="psD")
                        nc.tensor.matmul(
                            psd[0:HD, :],
                            lhsT=ones_row[0:1, 0:HD],
                            rhs=rden,
                            start=True,
                            stop=True,
                        )
                        denb = attntp.tile([HD, T], FP, tag="denb")
                        nc.vector.tensor_copy(out=denb, in_=psd[0:HD, :])
                        nc.vector.tensor_tensor(
                            ctxT[po : po + HD, oc, tb : tb + T],
                            psc[0:HD, :],
                            denb,
                            MUL,
                        )

                    if hp == 1:
                        # ---- output projection for this batch's tokens ----
                        for j in range(4 * b, 4 * b + 4):
                            pso = psB.tile([P, 512], FP, tag="psB")
                            for kc in range(2):
                                nc.tensor.matmul(
                                    pso[:, 0:D],
                                    lhsT=ctxT[:, kc, P * j : P * (j + 1)],
                                    rhs=wo_sb[:, kc, :],
                                    start=(kc == 0),
                                    stop=False,
                                )
                            nc.tensor.matmul(
                                pso[:, 0:D],
                                lhsT=ones_row,
                                rhs=bo_row,
                                start=False,
                                stop=True,
                            )
                            o_sb = work.tile([P, D], FP, tag="o_sb")
                            nc.vector.tensor_copy(out=o_sb, in_=pso[:, 0:D])
                            nc.sync.dma_start(
                                out=out_ext[P * j : P * (j + 1), :],
                                in_=o_sb,
                            )
    nc.finalize()
    return nc


def _get_nc():
    if "nc" not in _CACHE:
        _CACHE["nc"] = _build_nc()
    return _CACHE["nc"]


def _make_in_maps(inputs):
    x = np.asarray(inputs["inputs"], dtype=np.float32)  # [16, 512, 256]
    enc = _enc_table()

    def wtile(w):
        # W [o, i] -> W.T [i, o] -> [p, (c o)] with i = c*128 + p
        return (
            np.asarray(w, np.float32)
            .T.reshape(2, P, D)
            .transpose(1, 0, 2)
            .reshape(P, 512)
        )

    def coltile(v):
        return np.asarray(v, np.float32).reshape(2, P).T  # [p, c]

    consts = np.zeros((P, CONSTW), np.float32)
    for name, w in [
        ("wqT", inputs["Wq"]),
        ("wkT", inputs["Wk"]),
        ("wvT", inputs["Wv"]),
        ("woT", inputs["Wo"]),
        ("wposT", inputs["Wpos"]),
    ]:
        consts[:, W_OFF[name] : W_OFF[name] + 512] = wtile(w)
    encp = np.zeros((2, P, 64), np.float32)
    encp[:, :, 0:NR] = enc.T.reshape(2, P, NR)
    consts[:, ENC_OFF : ENC_OFF + 128] = encp.transpose(1, 0, 2).reshape(P, 128)
    consts[:, ID_OFF : ID_OFF + 128] = np.eye(P, dtype=np.float32)
    consts[0, ONES_OFF : ONES_OFF + P] = 1.0
    vecs = np.zeros((P, 2, 8), np.float32)
    vecs[:, :, 0] = coltile(inputs["ln_gamma"])
    vecs[:, :, 1] = coltile(inputs["ln_beta"])
    vecs[:, :, 2] = coltile(inputs["bq"])
    vecs[:, :, 3] = coltile(inputs["bk"])
    vecs[:, :, 4] = coltile(inputs["bpos"])
    vecs[:, :, 5] = coltile(np.asarray(inputs["u_bias"], np.float32).reshape(D))
    vecs[:, :, 6] = coltile(np.asarray(inputs["v_bias"], np.float32).reshape(D))
    consts[:, VEC_OFF : VEC_OFF + 16] = vecs.reshape(P, 16)

    rows = np.stack(
        [
            np.asarray(inputs["bv"], np.float32),
            np.asarray(inputs["bo"], np.float32),
        ]
    )
    common = {
        "consts": np.ascontiguousarray(consts),
        "rows": np.ascontiguousarray(rows),
    }
    in_maps = []
    for core in range(NCORES):
        m = dict(common)
        m["x"] = np.ascontiguousarray(
            x[BPC * core : BPC * (core + 1)].reshape(N, D)
        )
        in_maps.append(m)
    return in_maps


def run(inputs, trace=False):
    nc = _get_nc()
    in_maps = _make_in_maps(inputs)
    res = run_bass_kernel_spmd(
        nc, in_maps, core_ids=list(range(NCORES)), trace=trace
    )
    outs = [np.asarray(r["out"]) for r in res.results]
    full = np.concatenate(outs, axis=0).reshape(B, T, D).astype(np.float32)
    return full, res


def kernel(**inputs) -> np.ndarray:
    full, _ = run(inputs, trace=False)
    return full


# revision 65
# speedup vs baseline: 1.2287x; 1.0376x over previous
"""
Trainium2 Bass kernel for nn_Attention_6150393168649.

Transformer-XL-style relative-position attention, b=16 t=512 d=256 h=4 hd=64,
MAX_REL=30.  Data-parallel over batch across 8 NeuronCores (2 batches/core);
weights replicated.

Key algorithmic points (per core):
  - LayerNorm stats in token-major layout; gamma/beta application folded into
    the PSUM->SBUF epilogue of the x transposes.
  - All linears consume xT (features on partitions); weights arrive
    host-pre-transposed and pre-tiled (layout-only marshalling).
  - rel-pos: the [t,t,d] tensor has only 61 distinct rows ->
    posT = (enc @ Wpos.T).T projected on device (61 x 256).
    pos scores factor through exp:  attn = exp(qk/8) * m, with
    m[t,s] = exp((pos[t,clip(s-t)+30] - pos[t,0])/8)  (the per-row constant
    pos[t,0] is dropped -- softmax invariant).  m == 1 left of the band;
    m == w[t] right of the band; only a 61-wide band is nontrivial.
  - The diagonal band "skew" runs through a DRAM scratch: each row stores
    [left-pad=1.0 | exp-multiplier band x61 | right-pad=w[t]] at stride 318;
    a stride-317 strided read yields m[t, s] for a 188-wide strip per
    128-token chunk.  Clip values come free from the pads.
  - softmax denominators come free from a ones-column appended to v
    (contexts computed unnormalized, divided in ctxT space via a rank-1
    broadcast matmul of the reciprocal).
  - attn is transposed for the context matmul with PE transposes (bf16).
  - matmuls run in float32r (TensorE full rate at N>=256); the attention
    probability path (attn/m tables) is bf16.
"""

import math
import sys

import numpy as np

sys.path.insert(0, "/opt/trn_rl_repo")

import concourse.bass as bass  # noqa: E402
import concourse.mybir as mybir  # noqa: E402
import concourse.tile as tile  # noqa: E402
from concourse import bacc as _bacc  # noqa: E402
from concourse.bass_utils import run_bass_kernel_spmd  # noqa: E402

# Problem constants (hardcoded per instructions)
B = 16
T = 512
D = 256
H = 4
HD = 64
MAX_REL = 30
NR = 2 * MAX_REL + 1  # 61
NCORES = 8
BPC = B // NCORES  # batches per core
N = BPC * T  # local tokens per core (1024)
P = 128

# skew buffer geometry: row = [left-pad(128) | band(61) | right-pad(128) | 1]
PAD = 128
ROWW = PAD + NR + PAD + 1  # 318
NSKEW = 8  # rotating skew buffers (one per (batch, head): no WAR reuse)

FP = mybir.dt.float32
FR = mybir.dt.float32r
BF = mybir.dt.bfloat16

# consts block column offsets (in fp32 words per partition)
W_OFF = {"wqT": 0, "wkT": 512, "wvT": 1024, "woT": 1536, "wposT": 2048}
ENC_OFF = 2560  # [2, 64] -> 128 (61 used, zero-padded for even-N f32r)
ID_OFF = 2688  # [128]
VEC_OFF = 2816  # [2, 8] -> 16
ONES_OFF = 2832  # [128] row of ones (row 0)
CONSTW = 2960

_CACHE = {}


def _enc_table():
    """61 x 256 sinusoidal table over clipped relative distances (pure
    function of (t, d); mirrors reference._rel_pos_encodings rows)."""
    n = NR
    positions = np.arange(n, dtype=np.float32)[:, None]
    div_term = np.exp(
        np.arange(0, D, 2, dtype=np.float32) * (-math.log(10000.0) / D)
    )
    ang = positions * div_term  # [n, d/2]
    enc = np.stack([np.sin(ang), np.cos(ang)], axis=-1).reshape(n, D)
    return enc.astype(np.float32)  # [61, 256]


def _build_nc():
    # Bacc (not raw Bass): its compile() legalizes multi-wait instructions
    # into standalone event-semaphores (the raw ISA has one wait slot).
    nc = _bacc.Bacc(
        "TRN2", target_bir_lowering=False, debug=False, num_devices=NCORES
    )

    x_ext = nc.declare_dram_parameter("x", [N, D], FP, isOutput=False)
    c_ext = nc.declare_dram_parameter("consts", [P, CONSTW], FR, isOutput=False)
    r_ext = nc.declare_dram_parameter("rows", [2, D], FR, isOutput=False)
    out_ext = nc.declare_dram_parameter("out", [N, D], FP, isOutput=True)

    skew = nc.dram_tensor("skewbuf", [NSKEW, T, ROWW], BF)

    EXP = mybir.ActivationFunctionType.Exp
    SQRT = mybir.ActivationFunctionType.Sqrt
    MUL = mybir.AluOpType.mult
    ADD = mybir.AluOpType.add
    SUB = mybir.AluOpType.subtract
    AXX = mybir.AxisListType.X

    with nc.allow_low_precision(
        reason="float32r matmuls (32-bit) + bf16 attention probabilities"
    ), tile.TileContext(nc) as tc:
        with (
            tc.tile_pool(name="persist", bufs=1) as pers,
            tc.tile_pool(name="work", bufs=4) as work,
            tc.tile_pool(name="xcp", bufs=1) as xcp,
            tc.tile_pool(name="attnp", bufs=16) as attnp,
            tc.tile_pool(name="attntp", bufs=12) as attntp,
            tc.tile_pool(name="mtabp", bufs=4) as mtabp,
            tc.tile_pool(name="small", bufs=8) as small,
            tc.tile_pool(name="psA", bufs=2, space="PSUM") as psA,
            tc.tile_pool(name="psB", bufs=2, space="PSUM") as psB,
            tc.tile_pool(name="psP", bufs=1, space="PSUM") as psP,
            tc.tile_pool(name="psD", bufs=1, space="PSUM") as psD,
            tc.tile_pool(name="psC", bufs=2, space="PSUM") as psC,
        ):
            # ------------- constants (one DMA per weight block) -------------
            def wload(name):
                t_ = pers.tile([P, 2, D], FR, tag=f"w_{name}")
                nc.sync.dma_start(
                    out=t_,
                    in_=c_ext[:, W_OFF[name] : W_OFF[name] + 512].rearrange(
                        "p (c o) -> p c o", c=2
                    ),
                )
                return t_

            tail = pers.tile([P, CONSTW - ENC_OFF], FR, tag="ctail")
            nc.sync.dma_start(out=tail, in_=c_ext[:, ENC_OFF:])
            enc_sb = tail[:, 0:128].rearrange("p (c r) -> p c r", c=2)
            ident_sb = tail[:, ID_OFF - ENC_OFF : ID_OFF - ENC_OFF + 128]
            vecs_sb = tail[
                :, VEC_OFF - ENC_OFF : VEC_OFF - ENC_OFF + 16
            ].rearrange("p (c k) -> p c k", c=2)
            # vec k: 0 gamma, 1 beta, 2 bq, 3 bk, 4 bpos, 5 ub, 6 vb, 7 spare
            # (copy to a plain-fp32 tile: tensor_scalar wants fp32 scalars)
            vecs_fp = pers.tile([P, 2, 8], FP, tag="vecs_fp")
            nc.vector.tensor_copy(out=vecs_fp, in_=vecs_sb)
            gamma_col = vecs_fp[:, :, 0]
            beta_col = vecs_fp[:, :, 1]
            bk_col = vecs_fp[:, :, 3]
            bp_col = vecs_fp[:, :, 4]

            rows_sb = pers.tile([1, 2, D], FR, tag="rows")
            nc.sync.dma_start(
                out=rows_sb, in_=r_ext[:].rearrange("(o r) d -> o r d", o=1)
            )
            bv_row = rows_sb[:, 0, :]
            bo_row = rows_sb[:, 1, :]

            ident_bf = pers.tile([P, P], BF, tag="ident_bf")
            nc.vector.tensor_copy(out=ident_bf, in_=ident_sb)
            ones_row = tail[0:1, ONES_OFF - ENC_OFF : ONES_OFF - ENC_OFF + P]
            ones_pad = pers.tile([P, PAD], BF, tag="ones_pad")
            nc.vector.memset(ones_pad, 1.0)
            eps_t = pers.tile([P, 1], FP, tag="eps")
            nc.vector.memset(eps_t, 1e-5)

            bqu_col = pers.tile([P, 2], FP, tag="bqu")
            nc.vector.tensor_tensor(
                bqu_col, vecs_fp[:, :, 2], vecs_fp[:, :, 5], ADD
            )
            bqv_col = pers.tile([P, 2], FP, tag="bqv")
            nc.vector.tensor_tensor(
                bqv_col, vecs_fp[:, :, 2], vecs_fp[:, :, 6], ADD
            )

            # ------------- load x (two DMAs) + LayerNorm (two halves) -------
            nj = N // P
            hj = nj // 4
            x_all = pers.tile([P, nj, D], FP, tag="x_all")
            xhat = xcp.tile([P, nj, D], FR, tag="xc")
            sq = xcp.tile([P, nj, D], FP, tag="sq")
            xv = x_ext[:].rearrange("(j p) d -> p j d", p=P)
            for g in range(4):
                gs = slice(hj * g, hj * (g + 1))
                nc.sync.dma_start(out=x_all[:, gs, :], in_=xv[:, gs, :])
                s1 = small.tile([P, hj], FP, tag="s1")
                nc.vector.reduce_sum(out=s1, in_=x_all[:, gs, :], axis=AXX)
                mu = small.tile([P, hj], FP, tag="mu")
                nc.vector.tensor_scalar_mul(mu, s1, 1.0 / D)
                nc.vector.tensor_tensor(
                    xhat[:, gs, :],
                    x_all[:, gs, :],
                    mu[:, :, None].to_broadcast((P, hj, D)),
                    SUB,
                )
                var = small.tile([P, hj], FP, tag="var")
                for j in range(hj):
                    nc.scalar.activation(
                        out=sq[:, hj * g + j, :],
                        in_=xhat[:, hj * g + j, :],
                        func=mybir.ActivationFunctionType.Square,
                        accum_out=var[:, j : j + 1],
                    )
                std = small.tile([P, hj], FP, tag="std")
                nc.scalar.activation(
                    out=std, in_=var, func=SQRT, bias=eps_t[:, 0:1],
                    scale=1.0 / D,
                )
                rs = small.tile([P, hj], FP, tag="rs")
                nc.vector.reciprocal(out=rs, in_=std)
                nc.vector.tensor_tensor(
                    xhat[:, gs, :],
                    xhat[:, gs, :],
                    rs[:, :, None].to_broadcast((P, hj, D)),
                    MUL,
                )
            xhat_tiles = [xhat[:, j, :] for j in range(nj)]

            wq_sb = wload("wqT")
            wk_sb = wload("wkT")
            wv_sb = wload("wvT")
            wo_sb = wload("woT")
            wp_sb = wload("wposT")
            # prefill left pads of the skew buffers with 1.0 (once each)
            for bi in range(NSKEW):
                dst = bass.AP(
                    tensor=skew[bi].tensor,
                    offset=skew[bi].offset,
                    ap=[[ROWW, P], [ROWW * P, 4], [1, PAD]],
                )
                src = bass.AP(
                    tensor=ones_pad.tensor,
                    offset=ones_pad[:].offset,
                    ap=[list(ones_pad[:].ap[0]), [0, 4], [1, PAD]],
                )
                nc.sync.dma_start(out=dst, in_=src)

            # -------- transpose x -> xT (gamma/beta in the epilogue) --------
            xT = pers.tile([P, 2, N], FR, tag="xT")
            for c in range(2):
                for g in range(2):
                    ps = psA.tile([P, 512], FR, tag="psA")
                    for jj in range(4):
                        j = 4 * g + jj
                        nc.tensor.transpose(
                            ps[:, P * jj : P * (jj + 1)],
                            xhat_tiles[j][:, P * c : P * (c + 1)],
                            ident_sb,
                        )
                    nc.vector.tensor_scalar(
                        out=xT[:, c, 512 * g : 512 * (g + 1)],
                        in0=ps,
                        scalar1=gamma_col[:, c : c + 1],
                        scalar2=beta_col[:, c : c + 1],
                        op0=MUL,
                        op1=ADD,
                    )

            # ---------------- projections ----------------
            qTu = pers.tile([P, 2, N], FR, tag="qTu")
            qTv = pers.tile([P, 2, N], FR, tag="qTv")
            kT = pers.tile([P, 2, N], FR, tag="kT")
            for oc in range(2):
                for g in range(2):
                    psq = psA.tile([P, 512], FP, tag="psA")
                    psk = psB.tile([P, 512], FP, tag="psB")
                    for kc in range(2):
                        nc.tensor.matmul(
                            psq,
                            lhsT=wq_sb[:, kc, P * oc : P * (oc + 1)],
                            rhs=xT[:, kc, 512 * g : 512 * (g + 1)],
                            start=(kc == 0),
                            stop=(kc == 1),
                        )
                        nc.tensor.matmul(
                            psk,
                            lhsT=wk_sb[:, kc, P * oc : P * (oc + 1)],
                            rhs=xT[:, kc, 512 * g : 512 * (g + 1)],
                            start=(kc == 0),
                            stop=(kc == 1),
                        )
                    sl = (slice(None), oc, slice(512 * g, 512 * (g + 1)))
                    nc.vector.tensor_scalar_add(
                        qTu[sl], psq, bqu_col[:, oc : oc + 1]
                    )
                    nc.scalar.activation(
                        out=qTv[sl],
                        in_=psq,
                        func=mybir.ActivationFunctionType.Identity,
                        bias=bqv_col[:, oc : oc + 1],
                    )
                    nc.vector.tensor_scalar_add(
                        kT[sl], psk, bk_col[:, oc : oc + 1]
                    )

            # v in token-major layout with a ones column per head (bf16)
            v_sb = pers.tile([P, N // P, H, HD + 1], BF, tag="v_sb")
            nc.vector.memset(v_sb, 1.0)  # ones cols (rest overwritten)
            for j in range(N // P):
                psv = psB.tile([P, 512], FP, tag="psB")
                for kc in range(2):
                    nc.tensor.matmul(
                        psv[:, 0:D],
                        lhsT=xT[:, kc, P * j : P * (j + 1)],
                        rhs=wv_sb[:, kc, :],
                        start=(kc == 0),
                        stop=False,
                    )
                nc.tensor.matmul(
                    psv[:, 0:D],
                    lhsT=ones_row,
                    rhs=bv_row,
                    start=False,
                    stop=True,
                )
                nc.vector.tensor_copy(
                    out=v_sb[:, j, :, 0:HD],
                    in_=psv[:, 0:D].rearrange("p (h d) -> p h d", h=H),
                )

            # ---------------- posT = (enc @ Wpos.T).T + bpos ----------------
            posT = pers.tile([P, 2, 64], FR, tag="posT")
            for mc in range(2):
                psp = psP.tile([P, 4, 64], FP, tag="psP")
                for kc in range(2):
                    nc.tensor.matmul(
                        psp[:, 0, :],
                        lhsT=wp_sb[:, kc, P * mc : P * (mc + 1)],
                        rhs=enc_sb[:, kc, :],
                        start=(kc == 0),
                        stop=(kc == 1),
                    )
                nc.vector.tensor_scalar_add(
                    posT[:, mc, :], psp[:, 0, :], bp_col[:, mc : mc + 1]
                )
            # delta table: pos_r - pos_0 -> m = exp(qv . dpos / 8), no bias
            posTd = pers.tile([P, 2, 64], FR, tag="posTd")
            nc.vector.tensor_tensor(
                posTd,
                posT,
                posT[:, :, 0:1].to_broadcast(posT.shape),
                SUB,
            )

            # ------- m-tables + skew buffers for ALL heads (hoisted) -------
            # Only needs qTv/posTd; emitting all of it up front takes the
            # band-multiplier DMA round-trip off the attention critical path.
            all_wvals = {}
            all_strips = {}
            for b in range(BPC):
                for hp in range(2):
                    oc = hp
                    tb = T * b
                    psps = []
                    for hh in range(2):
                        psp_h = psP.tile([P, 4, 64], FP, tag="psP", name=f"psp_{b}_{hp}_{hh}")
                        psps.append(psp_h)
                    for t4 in range(4):
                        for hh in range(2):
                            po = HD * hh
                            nc.tensor.matmul(
                                psps[hh][:, t4, :],
                                lhsT=qTv[
                                    po : po + HD,
                                    oc,
                                    tb + P * t4 : tb + P * (t4 + 1),
                                ],
                                rhs=posTd[po : po + HD, oc, :],
                                start=True,
                                stop=True,
                            )
                    for hh in range(2):
                        bh = b * H + 2 * hp + hh
                        bi = bh % NSKEW
                        mtab = mtabp.tile([P, 4, 64], BF, tag="mtab")
                        wrep = mtabp.tile([P, 4, PAD], BF, tag="wrep")
                        wvals = small.tile([P, 4], FP, tag="wvals")
                        nc.scalar.activation(
                            out=mtab[:], in_=psps[hh][:], func=EXP, scale=0.125
                        )
                        nc.gpsimd.tensor_copy(
                            out=wvals, in_=mtab[:, :, NR - 1]
                        )
                        for t4 in range(4):
                            nc.gpsimd.tensor_scalar_mul(
                                wrep[:, t4, :], ones_pad, wvals[:, t4 : t4 + 1]
                            )
                        nc.sync.dma_start(
                            out=bass.AP(
                                tensor=skew[bi].tensor,
                                offset=skew[bi].offset + PAD,
                                ap=[[ROWW, P], [ROWW * P, 4], [1, NR]],
                            ),
                            in_=mtab[:, :, 0:NR],
                        )
                        nc.sync.dma_start(
                            out=bass.AP(
                                tensor=skew[bi].tensor,
                                offset=skew[bi].offset + PAD + NR,
                                ap=[[ROWW, P], [ROWW * P, 4], [1, PAD]],
                            ),
                            in_=wrep[:],
                        )
                        strip = work.tile(
                            [P, 4, 188], BF, tag=f"strip_{bh}", bufs=1
                        )
                        nc.sync.dma_start(
                            out=strip,
                            in_=bass.AP(
                                tensor=skew[bi].tensor,
                                offset=skew[bi].offset + PAD,
                                ap=[[ROWW - 1, P], [ROWW * P, 4], [1, 188]],
                            ),
                        )
                        all_wvals[bh] = wvals
                        all_strips[bh] = strip

            # ---------------- attention per (batch, head-pair) ----------------
            ctxT = pers.tile([P, 2, N], FR, tag="ctxT")
            for b in range(BPC):
                for hp in range(2):
                    oc = hp
                    tb = T * b
                    strips = [all_strips[b * H + 2 * hp + hh] for hh in range(2)]
                    wvalss = [all_wvals[b * H + 2 * hp + hh] for hh in range(2)]

                    # --- scores -> exp -> multiplier (pair-interleaved) ---
                    attns = [[], []]
                    for t4 in range(4):
                        T0 = P * t4
                        s_lo = max(0, T0 - 30)
                        s_hi = min(T, T0 + 30 + P)
                        c_lo = s_lo - (T0 - 30)
                        psss = []
                        for hh in range(2):
                            po = HD * hh
                            pss = psA.tile([P, 512], FP, tag="psA")
                            nc.tensor.matmul(
                                pss,
                                lhsT=qTu[
                                    po : po + HD,
                                    oc,
                                    tb + P * t4 : tb + P * (t4 + 1),
                                ],
                                rhs=kT[po : po + HD, oc, tb : tb + T],
                                start=True,
                                stop=True,
                            )
                            psss.append(pss)
                        for hh in range(2):
                            at = attnp.tile([P, T], BF, tag="attn")
                            nc.scalar.activation(
                                out=at, in_=psss[hh], func=EXP, scale=0.125
                            )
                            nc.vector.tensor_tensor(
                                at[:, s_lo:s_hi],
                                at[:, s_lo:s_hi],
                                strips[hh][:, t4, c_lo : c_lo + (s_hi - s_lo)],
                                MUL,
                            )
                            if s_hi < T:
                                nc.gpsimd.tensor_scalar_mul(
                                    at[:, s_hi:T],
                                    at[:, s_hi:T],
                                    wvalss[hh][:, t4 : t4 + 1],
                                )
                            attns[hh].append(at)

                    # --- transpose + context + normalize, per head ---
                    for hh in range(2):
                        h = 2 * hp + hh
                        po = HD * hh
                        attnTs = []
                        for s4 in range(4):
                            psat = psB.tile([P, 512], BF, tag="psB")
                            for t4 in range(4):
                                nc.tensor.transpose(
                                    psat[:, P * t4 : P * (t4 + 1)],
                                    attns[hh][t4][:, P * s4 : P * (s4 + 1)],
                                    ident_bf,
                                )
                            atT = attntp.tile([P, T], BF, tag="attnT")
                            nc.vector.tensor_copy(out=atT, in_=psat)
                            attnTs.append(atT)

                        psc = psC.tile([P, 512], FP, tag="psC")
                        for s4 in range(4):
                            j = 4 * b + s4
                            nc.tensor.matmul(
                                psc[0 : HD + 1, :],
                                lhsT=v_sb[:, j, h, :],
                                rhs=attnTs[s4],
                                start=(s4 == 0),
                                stop=(s4 == 3),
                            )
                        rden = small.tile([1, T], FR, tag="rden")
                        nc.vector.reciprocal(
                            out=rden, in_=psc[HD : HD + 1, :]
                        )
                        psd = psD.tile([P, 512], FP, tag="psD")
                        nc.tensor.matmul(
                            psd[0:HD, :],
                            lhsT=ones_row[0:1, 0:HD],
                            rhs=rden,
                            start=True,
                            stop=True,
                        )
                        denb = attntp.tile([HD, T], FP, tag="denb")
                        nc.vector.tensor_copy(out=denb, in_=psd[0:HD, :])
                        nc.vector.tensor_tensor(
                            ctxT[po : po + HD, oc, tb : tb + T],
                            psc[0:HD, :],
                            denb,
                            MUL,
                        )

                    if hp == 1:
                        # ---- output projection for this batch's tokens ----
                        for j in range(4 * b, 4 * b + 4):
                            pso = psB.tile([P, 512], FP, tag="psB")
                            for kc in range(2):
                                nc.tensor.matmul(
                                    pso[:, 0:D],
                                    lhsT=ctxT[:, kc, P * j : P * (j + 1)],
                                    rhs=wo_sb[:, kc, :],
                                    start=(kc == 0),
                                    stop=False,
                                )
                            nc.tensor.matmul(
                                pso[:, 0:D],
                                lhsT=ones_row,
                                rhs=bo_row,
                                start=False,
                                stop=True,
                            )
                            o_sb = work.tile([P, D], FP, tag="o_sb")
                            nc.vector.tensor_copy(out=o_sb, in_=pso[:, 0:D])
                            nc.sync.dma_start(
                                out=out_ext[P * j : P * (j + 1), :],
                                in_=o_sb,
                            )
    nc.finalize()
    return nc


def _get_nc():
    if "nc" not in _CACHE:
        _CACHE["nc"] = _build_nc()
    return _CACHE["nc"]


def _make_in_maps(inputs):
    x = np.asarray(inputs["inputs"], dtype=np.float32)  # [16, 512, 256]
    enc = _enc_table()

    def wtile(w):
        # W [o, i] -> W.T [i, o] -> [p, (c o)] with i = c*128 + p
        return (
            np.asarray(w, np.float32)
            .T.reshape(2, P, D)
            .transpose(1, 0, 2)
            .reshape(P, 512)
        )

    def coltile(v):
        return np.asarray(v, np.float32).reshape(2, P).T  # [p, c]

    consts = np.zeros((P, CONSTW), np.float32)
    for name, w in [
        ("wqT", inputs["Wq"]),
        ("wkT", inputs["Wk"]),
        ("wvT", inputs["Wv"]),
        ("woT", inputs["Wo"]),
        ("wposT", inputs["Wpos"]),
    ]:
        consts[:, W_OFF[name] : W_OFF[name] + 512] = wtile(w)
    encp = np.zeros((2, P, 64), np.float32)
    encp[:, :, 0:NR] = enc.T.reshape(2, P, NR)
    consts[:, ENC_OFF : ENC_OFF + 128] = encp.transpose(1, 0, 2).reshape(P, 128)
    consts[:, ID_OFF : ID_OFF + 128] = np.eye(P, dtype=np.float32)
    consts[0, ONES_OFF : ONES_OFF + P] = 1.0
    vecs = np.zeros((P, 2, 8), np.float32)
    vecs[:, :, 0] = coltile(inputs["ln_gamma"])
    vecs[:, :, 1] = coltile(inputs["ln_beta"])
    vecs[:, :, 2] = coltile(inputs["bq"])
    vecs[:, :, 3] = coltile(inputs["bk"])
    vecs[:, :, 4] = coltile(inputs["bpos"])
    vecs[:, :, 5] = coltile(np.asarray(inputs["u_bias"], np.float32).reshape(D))
    vecs[:, :, 6] = coltile(np.asarray(inputs["v_bias"], np.float32).reshape(D))
    consts[:, VEC_OFF : VEC_OFF + 16] = vecs.reshape(P, 16)

    rows = np.stack(
        [
            np.asarray(inputs["bv"], np.float32),
            np.asarray(inputs["bo"], np.float32),
        ]
    )
    common = {
        "consts": np.ascontiguousarray(consts),
        "rows": np.ascontiguousarray(rows),
    }
    in_maps = []
    for core in range(NCORES):
        m = dict(common)
        m["x"] = np.ascontiguousarray(
            x[BPC * core : BPC * (core + 1)].reshape(N, D)
        )
        in_maps.append(m)
    return in_maps


def run(inputs, trace=False):
    nc = _get_nc()
    in_maps = _make_in_maps(inputs)
    res = run_bass_kernel_spmd(
        nc, in_maps, core_ids=list(range(NCORES)), trace=trace
    )
    outs = [np.asarray(r["out"]) for r in res.results]
    full = np.concatenate(outs, axis=0).reshape(B, T, D).astype(np.float32)
    return full, res


def kernel(**inputs) -> np.ndarray:
    full, _ = run(inputs, trace=False)
    return full
